# revision 47
# baseline (speedup 1.0000x reference)
"""Trainium2 Bass kernel for the VQ codebook sampler problem.

Problem (hardcoded shapes):
  x:        [16, 4096, 1024] fp32
  codebook: [256, 1024] fp32
  d[b,c,t] = |cb_c|^2 - 2 cb_c.x_bt + |x_bt|^2          (squared L2)
  idx[b,c] = argmin_t d[b,c,t]
  gathered = x[b, idx[b,c], :]
  ste      = cb + (gathered - cb)      (forward value == gathered)
  loss     = mean((g-cb)^2) + mean((cb-g)^2)

Device computes, per (b, c): argmax_t dot[c,t] with
  dot[c,t] = cb_c . x_bt - |x_bt|^2 / 2
(equivalent to argmin_t d since |cb_c|^2 is constant per row and
 d = -2*dot + |cb_c|^2). The matmul runs in bf16 on the PE; the -xx/2
term is folded in as a rank-2 accumulate (ones[2,128] x [xx_hi; xx_lo]).

Sharding: data-parallel over batch, 2 batches per core on 8 cores;
codebook replicated. The device returns per-(b, c, t-segment) top-8
values + indices (DVE max8/find_index8); the host merges segments,
gathers rows, applies the straight-through estimator and the loss, and
re-resolves any row whose global top-2 margin is below a calibrated
threshold with an exact fp64 recompute. That margin test makes the
argmax provably exact for any dot-error bound E with THRESH >= 2E:
every token absent from the candidate list is bounded by the 2nd-best
candidate value, so margin > 2E guarantees the winner is the true
argmax. Near-tie rows (a few hundred out of 4096) cost one small fp64
GEMM on the host.

Layouts are chosen host-side so every device DMA is large and fully
contiguous per partition:
  xe [BPC, NTCH, 128, KT*TCH] : xe[b, tch, p, k*TCH+t]        = x[g, tch*TCH+t, k*128+p]
  cbt [128, KT*C]             : cbt[p, ct*KT*128 + k*128 + c'] = cb[ct*128+c', k*128+p]
  xxe [BPC, 2, T]             : hi/lo split of -|x|^2/2
"""

import os
from contextlib import ExitStack

import ml_dtypes
import numpy as np

import concourse.bacc as bacc
import concourse.mybir as mybir
import concourse.tile as tile
from concourse.bass_utils import run_bass_kernel_spmd

B, T, D, C = 16, 4096, 1024, 256
NCORES = 8
BPC = B // NCORES            # batches per core
KT = D // 128                # k tiles (8)
TCH = 1024                   # t chunk per DMA
NTCH = T // TCH              # 4
NPS = TCH // 512             # psum tiles per chunk (N=512 each)
SEGW = 512                   # top-8 segment width (one PSUM tile)
NSEG = T // SEGW             # 8
NCT = C // 128               # c tiles (2)

# matmul input dtype for x / codebook. bf16 halves HBM traffic and is the
# fastest PE dtype; its rounding error is fully guarded by the margin
# fallback (see module docstring).
_DT_NAME = os.environ.get("KERNEL_DT", "bf16")
MM_DT = {"bf16": mybir.dt.bfloat16,
         "fp16": mybir.dt.float16,
         "fp32r": mybir.dt.float32r,
         "fp32": mybir.dt.float32}[_DT_NAME]
# margin below which a row's argmax is re-resolved exactly on host
THRESH = {"bf16": 1.5, "fp16": 0.35, "fp32r": 0.25, "fp32": 0.02}[_DT_NAME]
NP_DT = {"bf16": ml_dtypes.bfloat16, "fp16": np.float16,
         "fp32r": np.float32, "fp32": np.float32}[_DT_NAME]

_CACHE = {}

LAST_RESULTS = None  # BassKernelResults of the last run (for profiling)
LAST_DIAG = None     # numerical diagnostics of the last run


def _build():
    nc = bacc.Bacc("TRN2", target_bir_lowering=False, debug=False,
                   num_devices=NCORES)
    xe = nc.dram_tensor("xe", [BPC, NTCH, 128, KT * TCH], MM_DT,
                        kind="ExternalInput").ap()
    # -|x_bt|^2/2 in fp32, pre-broadcast across the 128 partitions
    xxb = nc.dram_tensor("xxb", [BPC, 128, T], mybir.dt.float32,
                         kind="ExternalInput").ap()
    cbt = nc.dram_tensor("cbt", [128, KT * C], MM_DT, kind="ExternalInput").ap()
    # per-(batch, ctile): NSEG segments x (top8 values | top8 indices)
    oval = nc.dram_tensor("oval", [BPC, NCT, 128, NSEG * 8], mybir.dt.float32,
                          kind="ExternalOutput").ap()
    oidx = nc.dram_tensor("oidx", [BPC, NCT, 128, NSEG * 8], mybir.dt.uint32,
                          kind="ExternalOutput").ap()

    with tile.TileContext(nc) as tc, ExitStack() as ctx:
        const_pool = ctx.enter_context(tc.tile_pool(name="const", bufs=1))
        x_pool = ctx.enter_context(tc.tile_pool(name="x", bufs=4))
        xx_pool = ctx.enter_context(tc.tile_pool(name="xx", bufs=2))
        dot_pool = ctx.enter_context(tc.tile_pool(name="dot", bufs=6))
        out_pool = ctx.enter_context(tc.tile_pool(name="out", bufs=2))
        psum_pool = ctx.enter_context(tc.tile_pool(name="ps", bufs=8, space="PSUM"))

        cbt_sb = const_pool.tile([128, KT * C], MM_DT)

        for b in range(BPC):
            # xx rides the second HWDGE ring (otherwise idle until the
            # output DMAs) so it never delays the x chunks the PE needs
            xxbt = xx_pool.tile([128, T], mybir.dt.float32)
            nc.scalar.dma_start(xxbt[:], xxb[b])
            vts = [out_pool.tile([128, NSEG * 8], mybir.dt.float32,
                                 tag=f"vt{ct}", name=f"vt_b{b}_ct{ct}")
                   for ct in range(NCT)]
            its = [out_pool.tile([128, NSEG * 8], mybir.dt.uint32,
                                 tag=f"it{ct}", name=f"it_b{b}_ct{ct}")
                   for ct in range(NCT)]
            for tch in range(NTCH):
                xt = x_pool.tile([128, KT * TCH], MM_DT)
                # x chunks ride one FIFO ring in exact consumption order;
                # one ring drives all 16 SDMA engines, and FIFO order keeps
                # prefetch from starving the chunk the PE needs next. The
                # codebook halves are interleaved so the PE can start after
                # only cbt_ct0 + chunk 0.
                if b == 0 and tch == 0:
                    # interleave codebook halves with k-pair pieces of the
                    # first chunk: the PE starts after cbt_ct0 + k01 (0.75MB)
                    nc.sync.dma_start(cbt_sb[:, 0:KT * 128], cbt[:, 0:KT * 128])
                    for kp in range(KT // 2):
                        o = 2 * kp * TCH
                        nc.sync.dma_start(xt[:, o:o + 2 * TCH],
                                          xe[b, tch, :, o:o + 2 * TCH])
                        if kp == 0:
                            nc.sync.dma_start(cbt_sb[:, KT * 128:],
                                              cbt[:, KT * 128:])
                else:
                    nc.sync.dma_start(xt[:], xe[b, tch])
                for ct in range(NCT):
                    for ips in range(NPS):
                        t0 = tch * TCH + ips * 512        # global t offset
                        seg = t0 // SEGW
                        ps = psum_pool.tile([128, 512], mybir.dt.float32)
                        for k in range(KT):
                            nc.tensor.matmul(
                                ps[:],
                                cbt_sb[:, ct * KT * 128 + k * 128: ct * KT * 128 + k * 128 + 128],
                                xt[:, k * TCH + ips * 512: k * TCH + ips * 512 + 512],
                                start=(k == 0), stop=(k == KT - 1))
                        dot = dot_pool.tile([128, SEGW], mybir.dt.float32,
                                            tag="dot", name=f"dot_{b}_{tch}_{ct}_{ips}")
                        nc.scalar.copy(dot[:], ps[:])
                        # exact fp32 -xx/2 add on the otherwise-idle GpSimd
                        nc.gpsimd.tensor_add(dot[:], dot[:], xxbt[:, t0:t0 + 512])
                        # per-segment top8 right after the add: hides under
                        # the next group's matmuls, keeps the tail short
                        nc.vector.max(vts[ct][:, seg * 8:seg * 8 + 8], dot[:])
                        nc.vector.max_index(its[ct][:, seg * 8:seg * 8 + 8],
                                            vts[ct][:, seg * 8:seg * 8 + 8], dot[:])
            for ct in range(NCT):
                nc.scalar.dma_start(oval[b, ct], vts[ct][:])
                nc.scalar.dma_start(oidx[b, ct], its[ct][:])
    nc.compile()
    return nc


def _get_nc():
    if "nc" not in _CACHE:
        _CACHE["nc"] = _build()
    return _CACHE["nc"]


def _trunc12(v):
    """Zero the low 12 mantissa bits -> exactly representable in fp32r."""
    return (np.asarray(v, np.float32).view(np.uint32) & np.uint32(0xFFFFF000)).view(np.float32)


def _round_dt(v):
    """Round to a value exactly representable in the matmul input dtype."""
    if MM_DT == mybir.dt.float32r:
        return _trunc12(v)
    return np.asarray(v, NP_DT)


def kernel(x, codebook):
    global LAST_RESULTS, LAST_DIAG
    x = np.ascontiguousarray(np.asarray(x, dtype=np.float32))
    cb = np.ascontiguousarray(np.asarray(codebook, dtype=np.float32))
    assert x.shape == (B, T, D) and cb.shape == (C, D)

    nc = _get_nc()

    # ct-major codebook: cbt[p, ct*KT*128 + k*128 + c'] = cb[ct*128+c', k*128+p]
    cbt = np.ascontiguousarray(
        cb.astype(NP_DT).reshape(NCT, 128, KT, 128).transpose(3, 0, 2, 1)
    ).reshape(128, KT * C)
    x64 = x.astype(np.float64)
    xx64 = np.einsum("btd,btd->bt", x64, x64)
    xxm64 = -0.5 * xx64                                   # [B, T]
    xxm32 = xxm64.astype(np.float32)

    # xe[g, tch, p, k*TCH+t] = x[g, tch*TCH+t, k*128+p]
    xe_all = np.ascontiguousarray(
        x.astype(NP_DT).reshape(B, NTCH, TCH, KT, 128).transpose(0, 1, 4, 3, 2)
    ).reshape(B, NTCH, 128, KT * TCH)

    in_maps = []
    for core in range(NCORES):
        sl = slice(core * BPC, (core + 1) * BPC)
        xxb = np.ascontiguousarray(
            np.broadcast_to(xxm32[sl, None, :], (BPC, 128, T)))
        in_maps.append({"xe": xe_all[sl], "xxb": xxb, "cbt": cbt})

    trace = os.environ.get("KERNEL_TRACE", "0") == "1"
    res = run_bass_kernel_spmd(nc, in_maps, core_ids=list(range(NCORES)),
                               trace=trace)
    LAST_RESULTS = res

    # [B, C, NSEG, 8] candidate values and global indices
    vals = np.empty((B, C, NSEG, 8), np.float32)
    gidx = np.empty((B, C, NSEG, 8), np.int64)
    seg_off = (np.arange(NSEG) * SEGW)[None, None, :, None]
    for core in range(NCORES):
        r = res.results[core]
        v = r["oval"].reshape(BPC, C, NSEG, 8)
        i = r["oidx"].reshape(BPC, C, NSEG, 8).astype(np.int64)
        vals[core * BPC:(core + 1) * BPC] = v
        gidx[core * BPC:(core + 1) * BPC] = i + seg_off

    # merge segment candidates: flat order (seg-major, rank-minor) preserves
    # first-occurrence tie semantics
    vflat = vals.reshape(B, C, NSEG * 8)
    iflat = gidx.reshape(B, C, NSEG * 8)
    best = np.argmax(vflat, axis=2)
    idx = np.take_along_axis(iflat, best[:, :, None], axis=2)[:, :, 0]
    v0 = np.take_along_axis(vflat, best[:, :, None], axis=2)[:, :, 0]
    v2 = vflat.copy()
    np.put_along_axis(v2, best[:, :, None], -np.inf, axis=2)
    v1 = v2.max(axis=2)
    margin = v0 - v1

    # diagnostic: device top-1 value vs exact fp64 dot at the same index
    cb64 = cb.astype(np.float64)
    rows64 = np.take_along_axis(x64, idx[:, :, None], axis=1)
    dot_at = np.einsum("bcd,cd->bc", rows64, cb64) + np.take_along_axis(xxm64, idx, axis=1)
    dev_err = np.abs(v0.astype(np.float64) - dot_at)
    LAST_DIAG = {"max_dev_err": float(dev_err.max()),
                 "n_flagged": int((margin < THRESH).sum()),
                 "min_margin": float(margin.min())}

    # Exact fp64 re-resolve for rows whose top-2 margin is within the
    # reduced-precision error envelope.
    flagged = margin < THRESH
    if flagged.any():
        for b in np.unique(np.nonzero(flagged)[0]):
            cs = np.nonzero(flagged[b])[0]
            dot = cb64[cs] @ x64[b].T + xxm64[b][None, :]    # [nflag, T]
            idx[b, cs] = np.argmax(dot, axis=1)

    g = np.take_along_axis(x, idx[:, :, None], axis=1)    # [B, C, D]
    ste = cb[None, :, :] + (g - cb[None, :, :])           # fp32, same op order as reference
    diff = g.astype(np.float64) - cb[None].astype(np.float64)
    loss = np.float32(2.0 * np.mean(diff * diff))
    return ste, loss


# revision 52
# speedup vs baseline: 1.0327x; 1.0327x over previous
"""Trainium2 Bass kernel for the VQ codebook sampler problem.

Problem (hardcoded shapes):
  x:        [16, 4096, 1024] fp32
  codebook: [256, 1024] fp32
  d[b,c,t] = |cb_c|^2 - 2 cb_c.x_bt + |x_bt|^2          (squared L2)
  idx[b,c] = argmin_t d[b,c,t]
  gathered = x[b, idx[b,c], :]
  ste      = cb + (gathered - cb)      (forward value == gathered)
  loss     = mean((g-cb)^2) + mean((cb-g)^2)

Device computes, per (b, c): argmax_t dot[c,t] with
  dot[c,t] = cb_c . x_bt - |x_bt|^2 / 2
(equivalent to argmin_t d since |cb_c|^2 is constant per row and
 d = -2*dot + |cb_c|^2). The matmul runs in bf16 on the PE; the -xx/2
term is folded in as a rank-2 accumulate (ones[2,128] x [xx_hi; xx_lo]).

Sharding: data-parallel over batch, 2 batches per core on 8 cores;
codebook replicated. The device returns per-(b, c, t-segment) top-8
values + indices (DVE max8/find_index8); the host merges segments,
gathers rows, applies the straight-through estimator and the loss, and
re-resolves any row whose global top-2 margin is below a calibrated
threshold with an exact fp64 recompute. That margin test makes the
argmax provably exact for any dot-error bound E with THRESH >= 2E:
every token absent from the candidate list is bounded by the 2nd-best
candidate value, so margin > 2E guarantees the winner is the true
argmax. Near-tie rows (a few hundred out of 4096) cost one small fp64
GEMM on the host.

Layouts are chosen host-side so every device DMA is large and fully
contiguous per partition:
  xe [BPC, NTCH, 128, KT*TCH] : xe[b, tch, p, k*TCH+t]        = x[g, tch*TCH+t, k*128+p]
  cbt [128, KT*C]             : cbt[p, ct*KT*128 + k*128 + c'] = cb[ct*128+c', k*128+p]
  xxe [BPC, 2, T]             : hi/lo split of -|x|^2/2
"""

import os
from contextlib import ExitStack

import ml_dtypes
import numpy as np

import concourse.bacc as bacc
import concourse.mybir as mybir
import concourse.tile as tile
from concourse.bass_utils import run_bass_kernel_spmd

B, T, D, C = 16, 4096, 1024, 256
NCORES = 8
BPC = B // NCORES            # batches per core
KT = D // 128                # k tiles (8)
TCH = 1024                   # t chunk per DMA
NTCH = T // TCH              # 4
NPS = TCH // 512             # psum tiles per chunk (N=512 each)
SEGW = 512                   # top-8 segment width (one PSUM tile)
NSEG = T // SEGW             # 8
NCT = C // 128               # c tiles (2)

# matmul input dtype for x / codebook. bf16 halves HBM traffic and is the
# fastest PE dtype; its rounding error is fully guarded by the margin
# fallback (see module docstring).
_DT_NAME = os.environ.get("KERNEL_DT", "bf16")
MM_DT = {"bf16": mybir.dt.bfloat16,
         "fp16": mybir.dt.float16,
         "fp32r": mybir.dt.float32r,
         "fp32": mybir.dt.float32}[_DT_NAME]
# margin below which a row's argmax is re-resolved exactly on host
THRESH = {"bf16": 1.5, "fp16": 0.35, "fp32r": 0.25, "fp32": 0.02}[_DT_NAME]
NP_DT = {"bf16": ml_dtypes.bfloat16, "fp16": np.float16,
         "fp32r": np.float32, "fp32": np.float32}[_DT_NAME]

_CACHE = {}

LAST_RESULTS = None  # BassKernelResults of the last run (for profiling)
LAST_DIAG = None     # numerical diagnostics of the last run


def _build():
    nc = bacc.Bacc("TRN2", target_bir_lowering=False, debug=False,
                   num_devices=NCORES)
    xe = nc.dram_tensor("xe", [BPC, NTCH, 128, KT * TCH], MM_DT,
                        kind="ExternalInput").ap()
    # rows 0/1: hi/lo split of -|x_bt|^2/2, per batch
    xxe = nc.dram_tensor("xxe", [BPC, 2, T], MM_DT, kind="ExternalInput").ap()
    cbt = nc.dram_tensor("cbt", [128, KT * C], MM_DT, kind="ExternalInput").ap()
    # per-(batch, ctile): NSEG segments x (top8 values | top8 indices)
    oval = nc.dram_tensor("oval", [BPC, NCT, 128, NSEG * 8], mybir.dt.float32,
                          kind="ExternalOutput").ap()
    oidx = nc.dram_tensor("oidx", [BPC, NCT, 128, NSEG * 8], mybir.dt.uint32,
                          kind="ExternalOutput").ap()

    with tile.TileContext(nc) as tc, ExitStack() as ctx:
        const_pool = ctx.enter_context(tc.tile_pool(name="const", bufs=1))
        x_pool = ctx.enter_context(tc.tile_pool(name="x", bufs=4))
        xx_pool = ctx.enter_context(tc.tile_pool(name="xx", bufs=2))
        dot_pool = ctx.enter_context(tc.tile_pool(name="dot", bufs=6))
        out_pool = ctx.enter_context(tc.tile_pool(name="out", bufs=2))
        psum_pool = ctx.enter_context(tc.tile_pool(name="ps", bufs=8, space="PSUM"))

        cbt_sb = const_pool.tile([128, KT * C], MM_DT)
        if MM_DT == mybir.dt.float32r:
            ones_t = const_pool.tile([2, 128], mybir.dt.float32)
            nc.vector.memset(ones_t[:], 1.0)
            ones_r = ones_t[:].bitcast(MM_DT)
        else:
            ones_t = const_pool.tile([2, 128], MM_DT)
            nc.vector.memset(ones_t[:], 1.0)
            ones_r = ones_t[:]

        for b in range(BPC):
            # xx rides the second HWDGE ring (otherwise idle until the
            # output DMAs) so it never delays the x chunks the PE needs
            xxm = xx_pool.tile([2, T], MM_DT)
            nc.scalar.dma_start(xxm[:], xxe[b])
            vts = [out_pool.tile([128, NSEG * 8], mybir.dt.float32,
                                 tag=f"vt{ct}", name=f"vt_b{b}_ct{ct}")
                   for ct in range(NCT)]
            its = [out_pool.tile([128, NSEG * 8], mybir.dt.uint32,
                                 tag=f"it{ct}", name=f"it_b{b}_ct{ct}")
                   for ct in range(NCT)]
            for tch in range(NTCH):
                xt = x_pool.tile([128, KT * TCH], MM_DT)
                # x chunks ride one FIFO ring in exact consumption order;
                # one ring drives all 16 SDMA engines, and FIFO order keeps
                # prefetch from starving the chunk the PE needs next. The
                # codebook halves are interleaved so the PE can start after
                # only cbt_ct0 + chunk 0.
                if b == 0 and tch == 0:
                    # interleave codebook halves with k-pair pieces of the
                    # first chunk: the PE starts after cbt_ct0 + k01 (0.75MB)
                    nc.sync.dma_start(cbt_sb[:, 0:KT * 128], cbt[:, 0:KT * 128])
                    for kp in range(KT // 2):
                        o = 2 * kp * TCH
                        nc.sync.dma_start(xt[:, o:o + 2 * TCH],
                                          xe[b, tch, :, o:o + 2 * TCH])
                        if kp == 0:
                            nc.sync.dma_start(cbt_sb[:, KT * 128:],
                                              cbt[:, KT * 128:])
                else:
                    nc.sync.dma_start(xt[:], xe[b, tch])
                for ct in range(NCT):
                    for ips in range(NPS):
                        t0 = tch * TCH + ips * 512        # global t offset
                        seg = t0 // SEGW
                        ps = psum_pool.tile([128, 512], mybir.dt.float32)
                        for k in range(KT):
                            nc.tensor.matmul(
                                ps[:],
                                cbt_sb[:, ct * KT * 128 + k * 128: ct * KT * 128 + k * 128 + 128],
                                xt[:, k * TCH + ips * 512: k * TCH + ips * 512 + 512],
                                start=(k == 0), stop=False)
                        # rank-2 accumulate of the hi/lo -xx/2 rows
                        nc.tensor.matmul(ps[:], ones_r, xxm[:, t0:t0 + 512],
                                         start=False, stop=True)
                        dot = dot_pool.tile([128, SEGW], mybir.dt.float32,
                                            tag="dot", name=f"dot_{b}_{tch}_{ct}_{ips}")
                        nc.scalar.copy(dot[:], ps[:])
                        # per-segment top8 right after the copy: hides under
                        # the next group's matmuls, keeps the tail short
                        nc.vector.max(vts[ct][:, seg * 8:seg * 8 + 8], dot[:])
                        nc.vector.max_index(its[ct][:, seg * 8:seg * 8 + 8],
                                            vts[ct][:, seg * 8:seg * 8 + 8], dot[:])
            for ct in range(NCT):
                nc.scalar.dma_start(oval[b, ct], vts[ct][:])
                nc.scalar.dma_start(oidx[b, ct], its[ct][:])
    nc.compile()
    return nc


def _get_nc():
    if "nc" not in _CACHE:
        _CACHE["nc"] = _build()
    return _CACHE["nc"]


def _trunc12(v):
    """Zero the low 12 mantissa bits -> exactly representable in fp32r."""
    return (np.asarray(v, np.float32).view(np.uint32) & np.uint32(0xFFFFF000)).view(np.float32)


def _round_dt(v):
    """Round to a value exactly representable in the matmul input dtype."""
    if MM_DT == mybir.dt.float32r:
        return _trunc12(v)
    return np.asarray(v, NP_DT)


def kernel(x, codebook):
    global LAST_RESULTS, LAST_DIAG
    x = np.ascontiguousarray(np.asarray(x, dtype=np.float32))
    cb = np.ascontiguousarray(np.asarray(codebook, dtype=np.float32))
    assert x.shape == (B, T, D) and cb.shape == (C, D)

    nc = _get_nc()

    # ct-major codebook: cbt[p, ct*KT*128 + k*128 + c'] = cb[ct*128+c', k*128+p]
    cbt = np.ascontiguousarray(
        cb.astype(NP_DT).reshape(NCT, 128, KT, 128).transpose(3, 0, 2, 1)
    ).reshape(128, KT * C)
    x64 = x.astype(np.float64)
    xx64 = np.einsum("btd,btd->bt", x64, x64)
    xxm64 = -0.5 * xx64                                   # [B, T]
    xxh = _round_dt(xxm64)
    xxl = _round_dt(xxm64 - xxh.astype(np.float64))

    # xe[g, tch, p, k*TCH+t] = x[g, tch*TCH+t, k*128+p]
    xe_all = np.ascontiguousarray(
        x.astype(NP_DT).reshape(B, NTCH, TCH, KT, 128).transpose(0, 1, 4, 3, 2)
    ).reshape(B, NTCH, 128, KT * TCH)

    in_maps = []
    for core in range(NCORES):
        sl = slice(core * BPC, (core + 1) * BPC)
        xxe = np.ascontiguousarray(
            np.stack([np.stack([xxh[g], xxl[g]])
                      for g in range(core * BPC, (core + 1) * BPC)]))
        in_maps.append({"xe": xe_all[sl], "xxe": xxe, "cbt": cbt})

    trace = os.environ.get("KERNEL_TRACE", "0") == "1"
    res = run_bass_kernel_spmd(nc, in_maps, core_ids=list(range(NCORES)),
                               trace=trace)
    LAST_RESULTS = res

    # [B, C, NSEG, 8] candidate values and global indices
    vals = np.empty((B, C, NSEG, 8), np.float32)
    gidx = np.empty((B, C, NSEG, 8), np.int64)
    seg_off = (np.arange(NSEG) * SEGW)[None, None, :, None]
    for core in range(NCORES):
        r = res.results[core]
        v = r["oval"].reshape(BPC, C, NSEG, 8)
        i = r["oidx"].reshape(BPC, C, NSEG, 8).astype(np.int64)
        vals[core * BPC:(core + 1) * BPC] = v
        gidx[core * BPC:(core + 1) * BPC] = i + seg_off

    # merge segment candidates: flat order (seg-major, rank-minor) preserves
    # first-occurrence tie semantics
    vflat = vals.reshape(B, C, NSEG * 8)
    iflat = gidx.reshape(B, C, NSEG * 8)
    best = np.argmax(vflat, axis=2)
    idx = np.take_along_axis(iflat, best[:, :, None], axis=2)[:, :, 0]
    v0 = np.take_along_axis(vflat, best[:, :, None], axis=2)[:, :, 0]
    v2 = vflat.copy()
    np.put_along_axis(v2, best[:, :, None], -np.inf, axis=2)
    v1 = v2.max(axis=2)
    margin = v0 - v1

    # diagnostic: device top-1 value vs exact fp64 dot at the same index
    cb64 = cb.astype(np.float64)
    rows64 = np.take_along_axis(x64, idx[:, :, None], axis=1)
    dot_at = np.einsum("bcd,cd->bc", rows64, cb64) + np.take_along_axis(xxm64, idx, axis=1)
    dev_err = np.abs(v0.astype(np.float64) - dot_at)
    LAST_DIAG = {"max_dev_err": float(dev_err.max()),
                 "n_flagged": int((margin < THRESH).sum()),
                 "min_margin": float(margin.min())}

    # Exact fp64 re-resolve for rows whose top-2 margin is within the
    # reduced-precision error envelope.
    flagged = margin < THRESH
    if flagged.any():
        for b in np.unique(np.nonzero(flagged)[0]):
            cs = np.nonzero(flagged[b])[0]
            dot = cb64[cs] @ x64[b].T + xxm64[b][None, :]    # [nflag, T]
            idx[b, cs] = np.argmax(dot, axis=1)

    g = np.take_along_axis(x, idx[:, :, None], axis=1)    # [B, C, D]
    ste = cb[None, :, :] + (g - cb[None, :, :])           # fp32, same op order as reference
    diff = g.astype(np.float64) - cb[None].astype(np.float64)
    loss = np.float32(2.0 * np.mean(diff * diff))
    return ste, loss


# revision 54
# speedup vs baseline: 1.0364x; 1.0036x over previous
"""Trainium2 Bass kernel for the VQ codebook sampler problem.

Problem (hardcoded shapes):
  x:        [16, 4096, 1024] fp32
  codebook: [256, 1024] fp32
  d[b,c,t] = |cb_c|^2 - 2 cb_c.x_bt + |x_bt|^2          (squared L2)
  idx[b,c] = argmin_t d[b,c,t]
  gathered = x[b, idx[b,c], :]
  ste      = cb + (gathered - cb)      (forward value == gathered)
  loss     = mean((g-cb)^2) + mean((cb-g)^2)

Device computes, per (b, c): argmax_t dot[c,t] with
  dot[c,t] = cb_c . x_bt - |x_bt|^2 / 2
(equivalent to argmin_t d since |cb_c|^2 is constant per row and
 d = -2*dot + |cb_c|^2). The matmul runs in bf16 on the PE; the -xx/2
term is folded in as a rank-2 accumulate (ones[2,128] x [xx_hi; xx_lo]).

Sharding: data-parallel over batch, 2 batches per core on 8 cores;
codebook replicated. The device returns per-(b, c, t-segment) top-8
values + indices (DVE max8/find_index8); the host merges segments,
gathers rows, applies the straight-through estimator and the loss, and
re-resolves any row whose global top-2 margin is below a calibrated
threshold with an exact fp64 recompute. That margin test makes the
argmax provably exact for any dot-error bound E with THRESH >= 2E:
every token absent from the candidate list is bounded by the 2nd-best
candidate value, so margin > 2E guarantees the winner is the true
argmax. Near-tie rows (a few hundred out of 4096) cost one small fp64
GEMM on the host.

Layouts are chosen host-side so every device DMA is large and fully
contiguous per partition:
  xe [BPC, NTCH, 128, KT*TCH] : xe[b, tch, p, k*TCH+t]        = x[g, tch*TCH+t, k*128+p]
  cbt [128, KT*C]             : cbt[p, ct*KT*128 + k*128 + c'] = cb[ct*128+c', k*128+p]
  xxe [BPC, 2, T]             : hi/lo split of -|x|^2/2
"""

import os
from contextlib import ExitStack

import ml_dtypes
import numpy as np

import concourse.bacc as bacc
import concourse.mybir as mybir
import concourse.tile as tile
from concourse.bass_utils import run_bass_kernel_spmd

B, T, D, C = 16, 4096, 1024, 256
NCORES = 8
BPC = B // NCORES            # batches per core
KT = D // 128                # k tiles (8)
TCH = 1024                   # t chunk per DMA
NTCH = T // TCH              # 4
NPS = TCH // 512             # psum tiles per chunk (N=512 each)
SEGW = 512                   # top-8 segment width (one PSUM tile)
NSEG = T // SEGW             # 8
NCT = C // 128               # c tiles (2)

# matmul input dtype for x / codebook. bf16 halves HBM traffic and is the
# fastest PE dtype; its rounding error is fully guarded by the margin
# fallback (see module docstring).
_DT_NAME = os.environ.get("KERNEL_DT", "bf16")
MM_DT = {"bf16": mybir.dt.bfloat16,
         "fp16": mybir.dt.float16,
         "fp32r": mybir.dt.float32r,
         "fp32": mybir.dt.float32}[_DT_NAME]
# margin below which a row's argmax is re-resolved exactly on host
THRESH = {"bf16": 1.5, "fp16": 0.35, "fp32r": 0.25, "fp32": 0.02}[_DT_NAME]
NP_DT = {"bf16": ml_dtypes.bfloat16, "fp16": np.float16,
         "fp32r": np.float32, "fp32": np.float32}[_DT_NAME]

_CACHE = {}

LAST_RESULTS = None  # BassKernelResults of the last run (for profiling)
LAST_DIAG = None     # numerical diagnostics of the last run


def _build():
    nc = bacc.Bacc("TRN2", target_bir_lowering=False, debug=False,
                   num_devices=NCORES)
    xe = nc.dram_tensor("xe", [BPC, NTCH, 128, KT * TCH], MM_DT,
                        kind="ExternalInput").ap()
    # rows 0/1: hi/lo split of -|x_bt|^2/2, per batch
    xxe = nc.dram_tensor("xxe", [BPC, 2, T], MM_DT, kind="ExternalInput").ap()
    cbt = nc.dram_tensor("cbt", [128, KT * C], MM_DT, kind="ExternalInput").ap()
    # per-(batch, ctile): NSEG segments x (top8 values | top8 indices)
    oval = nc.dram_tensor("oval", [BPC, NCT, 128, NSEG * 8], mybir.dt.float32,
                          kind="ExternalOutput").ap()
    oidx = nc.dram_tensor("oidx", [BPC, NCT, 128, NSEG * 8], mybir.dt.uint32,
                          kind="ExternalOutput").ap()

    with tile.TileContext(nc) as tc, ExitStack() as ctx:
        const_pool = ctx.enter_context(tc.tile_pool(name="const", bufs=1))
        x_pool = ctx.enter_context(tc.tile_pool(name="x", bufs=4))
        xx_pool = ctx.enter_context(tc.tile_pool(name="xx", bufs=2))
        dot_pool = ctx.enter_context(tc.tile_pool(name="dot", bufs=6))
        out_pool = ctx.enter_context(tc.tile_pool(name="out", bufs=2))
        psum_pool = ctx.enter_context(tc.tile_pool(name="ps", bufs=8, space="PSUM"))

        cbt_sb = const_pool.tile([128, KT * C], MM_DT)
        if MM_DT == mybir.dt.float32r:
            ones_t = const_pool.tile([2, 128], mybir.dt.float32)
            nc.vector.memset(ones_t[:], 1.0)
            ones_r = ones_t[:].bitcast(MM_DT)
        else:
            ones_t = const_pool.tile([2, 128], MM_DT)
            nc.vector.memset(ones_t[:], 1.0)
            ones_r = ones_t[:]

        for b in range(BPC):
            # xx rides the second HWDGE ring (otherwise idle until the
            # output DMAs) so it never delays the x chunks the PE needs
            xxm = xx_pool.tile([2, T], MM_DT)
            nc.scalar.dma_start(xxm[:], xxe[b])
            vts = [out_pool.tile([128, NSEG * 8], mybir.dt.float32,
                                 tag=f"vt{ct}", name=f"vt_b{b}_ct{ct}")
                   for ct in range(NCT)]
            its = [out_pool.tile([128, NSEG * 8], mybir.dt.uint32,
                                 tag=f"it{ct}", name=f"it_b{b}_ct{ct}")
                   for ct in range(NCT)]
            for tch in range(NTCH):
                xt = x_pool.tile([128, KT * TCH], MM_DT)
                # x chunks ride one FIFO ring in exact consumption order;
                # one ring drives all 16 SDMA engines, and FIFO order keeps
                # prefetch from starving the chunk the PE needs next. The
                # codebook halves are interleaved so the PE can start after
                # only cbt_ct0 + chunk 0.
                if b == 0 and tch == 0:
                    # interleave codebook halves with k-pair pieces of the
                    # first chunk: the PE starts after cbt_ct0 + k01 (0.75MB)
                    nc.sync.dma_start(cbt_sb[:, 0:KT * 128], cbt[:, 0:KT * 128])
                    for kp in range(KT // 2):
                        o = 2 * kp * TCH
                        nc.sync.dma_start(xt[:, o:o + 2 * TCH],
                                          xe[b, tch, :, o:o + 2 * TCH])
                        if kp == 0:
                            nc.sync.dma_start(cbt_sb[:, KT * 128:],
                                              cbt[:, KT * 128:])
                else:
                    nc.sync.dma_start(xt[:], xe[b, tch])
                for ct in range(NCT):
                    # interleave the two psum tiles k-by-k so consecutive
                    # matmul pairs share the same lhsT: each LDWEIGHTS
                    # amortizes over 1024 streamed columns instead of 512
                    pss = [psum_pool.tile([128, 512], mybir.dt.float32, tag="ps",
                                          name=f"ps_{b}_{tch}_{ct}_{ips}")
                           for ips in range(NPS)]
                    for k in range(KT):
                        for ips in range(NPS):
                            nc.tensor.matmul(
                                pss[ips][:],
                                cbt_sb[:, ct * KT * 128 + k * 128: ct * KT * 128 + k * 128 + 128],
                                xt[:, k * TCH + ips * 512: k * TCH + ips * 512 + 512],
                                start=(k == 0), stop=False)
                    for ips in range(NPS):
                        t0 = tch * TCH + ips * 512        # global t offset
                        seg = t0 // SEGW
                        # rank-2 accumulate of the hi/lo -xx/2 rows
                        nc.tensor.matmul(pss[ips][:], ones_r, xxm[:, t0:t0 + 512],
                                         start=False, stop=True)
                        dot = dot_pool.tile([128, SEGW], mybir.dt.float32,
                                            tag="dot", name=f"dot_{b}_{tch}_{ct}_{ips}")
                        nc.scalar.copy(dot[:], pss[ips][:])
                        # per-segment top8 right after the copy: hides under
                        # the next group's matmuls, keeps the tail short
                        nc.vector.max(vts[ct][:, seg * 8:seg * 8 + 8], dot[:])
                        nc.vector.max_index(its[ct][:, seg * 8:seg * 8 + 8],
                                            vts[ct][:, seg * 8:seg * 8 + 8], dot[:])
            for ct in range(NCT):
                nc.scalar.dma_start(oval[b, ct], vts[ct][:])
                nc.scalar.dma_start(oidx[b, ct], its[ct][:])
    nc.compile()
    return nc


def _get_nc():
    if "nc" not in _CACHE:
        _CACHE["nc"] = _build()
    return _CACHE["nc"]


def _trunc12(v):
    """Zero the low 12 mantissa bits -> exactly representable in fp32r."""
    return (np.asarray(v, np.float32).view(np.uint32) & np.uint32(0xFFFFF000)).view(np.float32)


def _round_dt(v):
    """Round to a value exactly representable in the matmul input dtype."""
    if MM_DT == mybir.dt.float32r:
        return _trunc12(v)
    return np.asarray(v, NP_DT)


def kernel(x, codebook):
    global LAST_RESULTS, LAST_DIAG
    x = np.ascontiguousarray(np.asarray(x, dtype=np.float32))
    cb = np.ascontiguousarray(np.asarray(codebook, dtype=np.float32))
    assert x.shape == (B, T, D) and cb.shape == (C, D)

    nc = _get_nc()

    # ct-major codebook: cbt[p, ct*KT*128 + k*128 + c'] = cb[ct*128+c', k*128+p]
    cbt = np.ascontiguousarray(
        cb.astype(NP_DT).reshape(NCT, 128, KT, 128).transpose(3, 0, 2, 1)
    ).reshape(128, KT * C)
    x64 = x.astype(np.float64)
    xx64 = np.einsum("btd,btd->bt", x64, x64)
    xxm64 = -0.5 * xx64                                   # [B, T]
    xxh = _round_dt(xxm64)
    xxl = _round_dt(xxm64 - xxh.astype(np.float64))

    # xe[g, tch, p, k*TCH+t] = x[g, tch*TCH+t, k*128+p]
    xe_all = np.ascontiguousarray(
        x.astype(NP_DT).reshape(B, NTCH, TCH, KT, 128).transpose(0, 1, 4, 3, 2)
    ).reshape(B, NTCH, 128, KT * TCH)

    in_maps = []
    for core in range(NCORES):
        sl = slice(core * BPC, (core + 1) * BPC)
        xxe = np.ascontiguousarray(
            np.stack([np.stack([xxh[g], xxl[g]])
                      for g in range(core * BPC, (core + 1) * BPC)]))
        in_maps.append({"xe": xe_all[sl], "xxe": xxe, "cbt": cbt})

    trace = os.environ.get("KERNEL_TRACE", "0") == "1"
    res = run_bass_kernel_spmd(nc, in_maps, core_ids=list(range(NCORES)),
                               trace=trace)
    LAST_RESULTS = res

    # [B, C, NSEG, 8] candidate values and global indices
    vals = np.empty((B, C, NSEG, 8), np.float32)
    gidx = np.empty((B, C, NSEG, 8), np.int64)
    seg_off = (np.arange(NSEG) * SEGW)[None, None, :, None]
    for core in range(NCORES):
        r = res.results[core]
        v = r["oval"].reshape(BPC, C, NSEG, 8)
        i = r["oidx"].reshape(BPC, C, NSEG, 8).astype(np.int64)
        vals[core * BPC:(core + 1) * BPC] = v
        gidx[core * BPC:(core + 1) * BPC] = i + seg_off

    # merge segment candidates: flat order (seg-major, rank-minor) preserves
    # first-occurrence tie semantics
    vflat = vals.reshape(B, C, NSEG * 8)
    iflat = gidx.reshape(B, C, NSEG * 8)
    best = np.argmax(vflat, axis=2)
    idx = np.take_along_axis(iflat, best[:, :, None], axis=2)[:, :, 0]
    v0 = np.take_along_axis(vflat, best[:, :, None], axis=2)[:, :, 0]
    v2 = vflat.copy()
    np.put_along_axis(v2, best[:, :, None], -np.inf, axis=2)
    v1 = v2.max(axis=2)
    margin = v0 - v1

    # diagnostic: device top-1 value vs exact fp64 dot at the same index
    cb64 = cb.astype(np.float64)
    rows64 = np.take_along_axis(x64, idx[:, :, None], axis=1)
    dot_at = np.einsum("bcd,cd->bc", rows64, cb64) + np.take_along_axis(xxm64, idx, axis=1)
    dev_err = np.abs(v0.astype(np.float64) - dot_at)
    LAST_DIAG = {"max_dev_err": float(dev_err.max()),
                 "n_flagged": int((margin < THRESH).sum()),
                 "min_margin": float(margin.min())}

    # Exact fp64 re-resolve for rows whose top-2 margin is within the
    # reduced-precision error envelope.
    flagged = margin < THRESH
    if flagged.any():
        for b in np.unique(np.nonzero(flagged)[0]):
            cs = np.nonzero(flagged[b])[0]
            dot = cb64[cs] @ x64[b].T + xxm64[b][None, :]    # [nflag, T]
            idx[b, cs] = np.argmax(dot, axis=1)

    g = np.take_along_axis(x, idx[:, :, None], axis=1)    # [B, C, D]
    ste = cb[None, :, :] + (g - cb[None, :, :])           # fp32, same op order as reference
    diff = g.astype(np.float64) - cb[None].astype(np.float64)
    loss = np.float32(2.0 * np.mean(diff * diff))
    return ste, loss
